# revision 3
# baseline (speedup 1.0000x reference)
"""Trainium2 Bass kernel for CorrCosine cost volumes.

Inputs (full): ref_features, cur_features [8, 256, 64, 64] f32.
out[b, hc, wc, hr, wr] = <cur_n[b, :, hc, wc], ref_n[b, :, hr, wr]>
where *_n are channel-L2-normalized features.

Sharding: data-parallel over batch B=8 across the 8 NeuronCores — each core
computes one batch's full [4096, 4096] cosine-similarity matrix:
  sim = (cur / |cur|_C).T @ (ref / |ref|_C)   with K = C = 256.

Per-core structure (Tile framework), bf16 pipeline:
  - host converts both inputs to bf16; the device loads them as-is
    (4 MB instead of 8 MB HBM input traffic per core). bf16 matmuls run
    the PE at 1 cycle/row unconditionally (f32r's full-speed path is
    conditional on moving>=256 and pstate; bf16 is not).
  - ref slices: square (ACT, f32r out) -> one all-ones [128,128] f32r matmul
    per K-chunk, which computes the partition-reduce AND replicates the
    sums across all partitions in one shot -> PSUM sqrt (ACT) ->
    reciprocal (DVE) -> ref_n scaled in place (bf16).
  - cur stays unnormalized; inverse norms are reduced into output-row layout
    [128, 32] via N=1 matmuls and folded into the mandatory PSUM->SBUF
    copies (ACT activation scale / DVE tensor_scalar), which also cast the
    f32 PSUM result to bf16.
  - main loop per m-chunk: n-groups of 3 with k-outer matmul order so each
    cur stationary tile is loaded 6x instead of 16x per chunk (LD_WEIGHTS
    is unmodeled in CoreSim but real on HW), PSUM double-buffered across
    groups, fused scaled copies alternating ScalarE/VectorE, ~3 MB bf16
    output DMAs with ramped group sizes, alternating between the SP/HWDGE
    and Pool/SWDGE queues.
  - output written bf16 (32 MB instead of 64 MB per core), upcast to f32
    on the host after gather. Total HBM traffic 36 MB/core vs 72 MB f32.
  - rel-err budget: bf16 inputs + bf16 matmul + bf16 output round-off
    ~3e-3 overall, well inside the 2e-2 gate.

loop_iters>1 wraps the body in a tc.For_i hardware loop for timing: device
exec then spans hundreds of ms, which is measurable through the axon tunnel
(single-shot exec hides entirely under the ~90 ms dispatch latency).
"""

import numpy as np

import concourse.bass as bass
import concourse.mybir as mybir
import concourse.tile as tile
from concourse import bacc, bass_utils

B, C, H, W = 8, 256, 64, 64
HW = H * W           # 4096 pixels
KP = 128             # partitions per K-chunk
NK = C // KP         # 2 K-chunks
MT = 128             # output partition tile (cur pixels)
NT = 512             # output free tile (ref pixels) = one f32 PSUM bank
NM = HW // MT        # 32 m-chunks
NN = HW // NT        # 8 n-tiles
NG = 3               # n-tiles per k-outer group (PSUM live tiles)
MO = 3               # m-chunks per output DMA (3 MB bf16 per dma_start)
MPS = NT // MT       # m-chunks per 512-pixel input slice = 4

F32 = mybir.dt.float32
F32R = mybir.dt.float32r
BF16 = mybir.dt.bfloat16
ACTF = mybir.ActivationFunctionType


def _kernel_body(tc, cur, ref, sim, loop_iters=1):
    nc = tc.nc
    with (
        tc.tile_pool(name="pers", bufs=1) as pers,
        tc.tile_pool(name="sqp", bufs=2) as sqp,
        tc.tile_pool(name="rowp", bufs=2) as rowp,
        tc.tile_pool(name="outp", bufs=2) as outp,
        tc.tile_pool(name="psmm", bufs=5, space=bass.MemorySpace.PSUM) as psmm,
        tc.tile_pool(name="pspre", bufs=2, space=bass.MemorySpace.PSUM) as pspre,
        tc.tile_pool(name="pscol", bufs=1, space=bass.MemorySpace.PSUM) as pscol,
    ):
        ones_col = pers.tile([KP, 1], F32, tag="ones_col")
        nc.vector.memset(ones_col, 1.0)
        # all-ones [128, 128] f32r stationary operand: ones_mat.T @ sq gives
        # the per-column sums replicated across all 128 partitions, fusing
        # the partition-reduce and the broadcast into one full-speed matmul
        ones_f32 = pers.tile([KP, KP], F32, tag="ones_f32")
        nc.vector.memset(ones_f32, 1.0)
        ones_mat = pers.tile([KP, KP], F32R, tag="ones_mat")
        nc.scalar.copy(ones_mat, ones_f32)

        cur_fr = pers.tile([KP, NK, HW], BF16, tag="cur_fr")
        ref_n = [
            pers.tile([KP, NK, NT], BF16, tag=f"ref_n{n}", name=f"ref_n{n}")
            for n in range(NN)
        ]
        inv_cur = pers.tile([KP, NM], F32, tag="inv_cur")

        cur_r = cur.rearrange("(k p) n -> p k n", p=KP)
        ref_r = ref.rearrange("(k p) n -> p k n", p=KP)
        sim_pm = sim.rearrange("(mm p) n -> p mm n", p=KP)

        def body():
            # input DMAs: all of ref first (every output tile reads all of
            # ref), cur slice 0 early (first m-chunks + inv_cur), rest of cur
            for n in range(NN):
                sl = slice(n * NT, (n + 1) * NT)
                nc.gpsimd.dma_start(out=ref_n[n], in_=ref_r[:, :, sl])
                if n == 0:
                    nc.gpsimd.dma_start(out=cur_fr[:, :, sl], in_=cur_r[:, :, sl])
            for n in range(1, NN):
                sl = slice(n * NT, (n + 1) * NT)
                nc.gpsimd.dma_start(out=cur_fr[:, :, sl], in_=cur_r[:, :, sl])

            def ref_chain(n):
                sq = sqp.tile([KP, NK, NT], F32R, tag="sq_r", name="sq_r")
                nc.scalar.activation(sq, ref_n[n], ACTF.Square)
                pb = pspre.tile([KP, NT], F32, tag="pre_bc", name="pb")
                for k in range(NK):
                    nc.tensor.matmul(
                        pb, ones_mat, sq[:, k, :], start=(k == 0), stop=(k == NK - 1)
                    )
                nc.scalar.activation(pb, pb, ACTF.Sqrt)
                inv128 = rowp.tile([KP, NT], F32, tag="inv128", name="inv128")
                nc.vector.reciprocal(inv128, pb)
                for k in range(NK):
                    nc.vector.tensor_mul(ref_n[n][:, k, :], ref_n[n][:, k, :], inv128)

            def cur_chain(n):
                sl = slice(n * NT, (n + 1) * NT)
                csq = sqp.tile([KP, NK, NT], F32, tag="sq_c", name="sq_c")
                if n % 2 == 0:
                    nc.vector.tensor_mul(csq, cur_fr[:, :, sl], cur_fr[:, :, sl])
                else:
                    nc.scalar.activation(csq, cur_fr[:, :, sl], ACTF.Square)
                pcol = pscol.tile([KP, MPS], F32, tag="pre_col", name="pcol")
                for j in range(MPS):
                    for k in range(NK):
                        nc.tensor.matmul(
                            pcol[:, j:j + 1],
                            csq[:, k, j * MT:(j + 1) * MT],
                            ones_col,
                            start=(k == 0),
                            stop=(k == NK - 1),
                        )
                ncur = rowp.tile([KP, MPS], F32, tag="ncur", name="ncur")
                nc.scalar.activation(ncur, pcol, ACTF.Sqrt)
                nc.vector.reciprocal(inv_cur[:, n * MPS:(n + 1) * MPS], ncur)

            def out_group(mo, msz, gi):
                out_sb = outp.tile([KP, MO, HW], BF16, tag="out", name="out_sb")
                for mi in range(msz):
                    m = mo + mi
                    # k-outer over n-groups: one stationary cur tile serves
                    # NG consecutive n-tiles before swapping
                    for g0 in range(0, NN, NG):
                        gn = range(g0, min(g0 + NG, NN))
                        ps = {
                            n: psmm.tile([KP, NT], F32, tag="mm", name="ps")
                            for n in gn
                        }
                        for k in range(NK):
                            for n in gn:
                                nc.tensor.matmul(
                                    ps[n],
                                    cur_fr[:, k, m * MT:(m + 1) * MT],
                                    ref_n[n][:, k, :],
                                    start=(k == 0),
                                    stop=(k == NK - 1),
                                )
                        for n in gn:
                            dst = out_sb[:, mi, n * NT:(n + 1) * NT]
                            if n % 2 == 0:
                                nc.scalar.mul(dst, ps[n], inv_cur[:, m:m + 1])
                            else:
                                nc.vector.tensor_scalar_mul(
                                    dst, ps[n], inv_cur[:, m:m + 1]
                                )
                # alternate the issuing queue (SP HWDGE / Pool SWDGE) so each
                # DMA's descriptor-gen overhead hides under the other's
                # in-flight transfer
                eng = nc.sync if gi % 2 == 0 else nc.gpsimd
                eng.dma_start(out=sim_pm[:, mo:mo + msz, :], in_=out_sb[:, :msz, :])

            # ref chains first (every output column needs all of ref);
            # cur chains 0-1 cover the first 8 m-chunks, the rest are
            # emitted after the first output groups so the first output
            # copies don't queue behind them on DVE/ACT.
            for n in range(NN):
                ref_chain(n)
                if n < 2:
                    cur_chain(n)

            groups = [1, 2, 3, 3] + [3] * 7 + [2]
            mo = 0
            for gi, msz in enumerate(groups):
                out_group(mo, msz, gi)
                mo += msz
                if gi == 2:
                    for n in range(2, NN):
                        cur_chain(n)
            assert mo == NM

        if loop_iters == 1:
            body()
        else:
            with tc.For_i(0, loop_iters, 1):
                body()


_NC_CACHE = {}


def _np_bf16(x):
    return np.asarray(x).astype(mybir.dt.np(BF16))


def _timing_input_arrays(cur, ref):
    """Map device-input tensor names -> per-batch host arrays for test.py."""
    return {"cur": _np_bf16(cur), "ref": _np_bf16(ref)}


def _get_nc(loop_iters=1):
    key = ("nc", loop_iters)
    if key not in _NC_CACHE:
        nc = bacc.Bacc("TRN2", target_bir_lowering=False, debug=False)
        cur_d = nc.dram_tensor("cur", [C, HW], BF16, kind="ExternalInput")
        ref_d = nc.dram_tensor("ref", [C, HW], BF16, kind="ExternalInput")
        sim_d = nc.dram_tensor("sim", [HW, HW], BF16, kind="ExternalOutput")
        with tile.TileContext(nc) as tc:
            _kernel_body(tc, cur_d.ap(), ref_d.ap(), sim_d.ap(), loop_iters=loop_iters)
        nc.compile()
        _NC_CACHE[key] = nc
    return _NC_CACHE[key]


def kernel(ref_features, cur_features, _run_kwargs=None):
    ref_np = _np_bf16(
        np.ascontiguousarray(np.asarray(ref_features, dtype=np.float32)).reshape(
            B, C, HW
        )
    )
    cur_np = _np_bf16(
        np.ascontiguousarray(np.asarray(cur_features, dtype=np.float32)).reshape(
            B, C, HW
        )
    )
    nc = _get_nc()
    in_maps = [{"cur": cur_np[b], "ref": ref_np[b]} for b in range(B)]
    res = bass_utils.run_bass_kernel_spmd(
        nc, in_maps, core_ids=list(range(B)), **(_run_kwargs or {})
    )
    out = np.stack(
        [np.asarray(res.results[b]["sim"]).astype(np.float32) for b in range(B)],
        axis=0,
    )
    if _run_kwargs is not None:
        _NC_CACHE["last_results"] = res
    return out.reshape(B, H, W, H, W)


# revision 4
# speedup vs baseline: 1.0451x; 1.0451x over previous
"""Trainium2 Bass kernel for CorrCosine cost volumes.

Inputs (full): ref_features, cur_features [8, 256, 64, 64] f32.
out[b, hc, wc, hr, wr] = <cur_n[b, :, hc, wc], ref_n[b, :, hr, wr]>
where *_n are channel-L2-normalized features.

Sharding: data-parallel over batch B=8 across the 8 NeuronCores — each core
computes one batch's full [4096, 4096] cosine-similarity matrix:
  sim = (cur / |cur|_C).T @ (ref / |ref|_C)   with K = C = 256.

Per-core structure (Tile framework), bf16 pipeline (measured rates from
For_i microbenchmarks on this HW: PE 512 matmuls 113 us, 36 MB DMA 108 us
(~333 GB/s), 128 paired PSUM->SBUF copies 77 us):
  - host converts both inputs to bf16 (4 MB in); output written bf16
    (32 MB out), upcast to f32 on the host. bf16 matmuls run 1 cycle/row
    at 2.4 GHz unconditionally.
  - fill phase: ref slices stream on the HWDGE/sync queue, cur slices on
    the SWDGE/gpsimd queue in parallel; all 16 norm chains run here while
    ACT/DVE/PE are otherwise idle (the ones-matmul partition-reduce keeps
    PE warm through the fill, avoiding a pstate drop).
  - main loop: per m-chunk 4 PSUM pair-tiles [128, 1024] (2 banks each,
    psmm bufs=4 = all 8 banks -> elasticity so PE never waits on copy
    drain), 4 matmuls per pair, paired scaled copies alternating
    ScalarE/VectorE (scale = inv_cur, also casts f32->bf16), 3 MB output
    DMAs alternating sync/gpsimd queues.
  - norm-phase PSUM (pre-broadcast + column sums) is carved out of the
    same psmm pool (halves of pair tiles) so the main loop gets all 8
    banks the moment the fill phase drains.
  - rel-err ~3e-3 overall vs the 2e-2 gate.

loop_iters>1 wraps the body in a tc.For_i hardware loop for timing: device
exec then spans hundreds of ms, which is measurable through the axon tunnel
(single-shot exec hides entirely under the ~90 ms dispatch latency).
"""

import numpy as np

import concourse.bass as bass
import concourse.mybir as mybir
import concourse.tile as tile
from concourse import bacc, bass_utils

B, C, H, W = 8, 256, 64, 64
HW = H * W           # 4096 pixels
KP = 128             # partitions per K-chunk
NK = C // KP         # 2 K-chunks
MT = 128             # output partition tile (cur pixels)
NT = 512             # output free tile (ref pixels) = one f32 PSUM bank
NM = HW // MT        # 32 m-chunks
NN = HW // NT        # 8 n-tiles
NP = NN // 2         # 4 n-pairs per m-chunk
MO = 3               # m-chunks per output DMA (3 MB bf16 per dma_start)
MPS = NT // MT       # m-chunks per 512-pixel input slice = 4

F32 = mybir.dt.float32
F32R = mybir.dt.float32r
BF16 = mybir.dt.bfloat16
ACTF = mybir.ActivationFunctionType


def _kernel_body(tc, cur, ref, sim, loop_iters=1):
    nc = tc.nc
    with (
        tc.tile_pool(name="pers", bufs=1) as pers,
        tc.tile_pool(name="sqp", bufs=2) as sqp,
        tc.tile_pool(name="rowp", bufs=2) as rowp,
        tc.tile_pool(name="outp", bufs=2) as outp,
        tc.tile_pool(name="psmm", bufs=4, space=bass.MemorySpace.PSUM) as psmm,
    ):
        ones_col = pers.tile([KP, 1], F32, tag="ones_col")
        nc.vector.memset(ones_col, 1.0)
        # all-ones [128, 128] f32r stationary operand: ones_mat.T @ sq gives
        # the per-column sums replicated across all 128 partitions, fusing
        # the partition-reduce and the broadcast into one full-speed matmul
        ones_f32 = pers.tile([KP, KP], F32, tag="ones_f32")
        nc.vector.memset(ones_f32, 1.0)
        ones_mat = pers.tile([KP, KP], F32R, tag="ones_mat")
        nc.scalar.copy(ones_mat, ones_f32)

        cur_fr = pers.tile([KP, NK, HW], BF16, tag="cur_fr")
        ref_n = [
            pers.tile([KP, NK, NT], BF16, tag=f"ref_n{n}", name=f"ref_n{n}")
            for n in range(NN)
        ]
        inv_cur = pers.tile([KP, NM], F32, tag="inv_cur")

        cur_r = cur.rearrange("(k p) n -> p k n", p=KP)
        ref_r = ref.rearrange("(k p) n -> p k n", p=KP)
        sim_pm = sim.rearrange("(mm p) n -> p mm n", p=KP)

        def body():
            # fill: ref slices on the HWDGE/sync queue, cur on SWDGE/gpsimd
            # (the output DMAs start ~25 us later, so both queues are free)
            for n in range(NN):
                sl = slice(n * NT, (n + 1) * NT)
                nc.sync.dma_start(out=ref_n[n], in_=ref_r[:, :, sl])
                nc.gpsimd.dma_start(out=cur_fr[:, :, sl], in_=cur_r[:, :, sl])

            def ref_chain(n):
                sq = sqp.tile([KP, NK, NT], F32R, tag="sq_r", name="sq_r")
                nc.scalar.activation(sq, ref_n[n], ACTF.Square)
                # norm-phase PSUM lives in half of a psmm pair tile so the
                # main loop can use all 8 banks
                pb = psmm.tile([KP, 2 * NT], F32, tag="mm", name="pb")[:, :NT]
                for k in range(NK):
                    nc.tensor.matmul(
                        pb, ones_mat, sq[:, k, :], start=(k == 0), stop=(k == NK - 1)
                    )
                nc.scalar.activation(pb, pb, ACTF.Sqrt)
                inv128 = rowp.tile([KP, NT], F32, tag="inv128", name="inv128")
                nc.vector.reciprocal(inv128, pb)
                for k in range(NK):
                    nc.vector.tensor_mul(ref_n[n][:, k, :], ref_n[n][:, k, :], inv128)

            def cur_chain(n):
                sl = slice(n * NT, (n + 1) * NT)
                csq = sqp.tile([KP, NK, NT], F32, tag="sq_c", name="sq_c")
                if n % 2 == 0:
                    nc.vector.tensor_mul(csq, cur_fr[:, :, sl], cur_fr[:, :, sl])
                else:
                    nc.scalar.activation(csq, cur_fr[:, :, sl], ACTF.Square)
                pcol = psmm.tile([KP, 2 * NT], F32, tag="mm", name="pcol")[:, :MPS]
                for j in range(MPS):
                    for k in range(NK):
                        nc.tensor.matmul(
                            pcol[:, j:j + 1],
                            csq[:, k, j * MT:(j + 1) * MT],
                            ones_col,
                            start=(k == 0),
                            stop=(k == NK - 1),
                        )
                ncur = rowp.tile([KP, MPS], F32, tag="ncur", name="ncur")
                nc.scalar.activation(ncur, pcol, ACTF.Sqrt)
                nc.vector.reciprocal(inv_cur[:, n * MPS:(n + 1) * MPS], ncur)

            # all norm chains in the fill phase, interleaved so each starts
            # as its input slice lands
            for n in range(NN):
                ref_chain(n)
                cur_chain(n)

            def out_group(mo, msz, gi):
                out_sb = outp.tile([KP, MO, HW], BF16, tag="out", name="out_sb")
                for mi in range(msz):
                    m = mo + mi
                    for p in range(NP):
                        ps2 = psmm.tile([KP, 2 * NT], F32, tag="mm", name="ps2")
                        for ni in range(2):
                            n = 2 * p + ni
                            for k in range(NK):
                                nc.tensor.matmul(
                                    ps2[:, ni * NT:(ni + 1) * NT],
                                    cur_fr[:, k, m * MT:(m + 1) * MT],
                                    ref_n[n][:, k, :],
                                    start=(k == 0),
                                    stop=(k == NK - 1),
                                )
                        dst = out_sb[:, mi, 2 * p * NT:2 * (p + 1) * NT]
                        if p % 2 == 0:
                            nc.scalar.mul(dst, ps2, inv_cur[:, m:m + 1])
                        else:
                            nc.vector.tensor_scalar_mul(dst, ps2, inv_cur[:, m:m + 1])
                # alternate the issuing queue (SP HWDGE / Pool SWDGE) so each
                # DMA's descriptor-gen overhead hides under the other's
                # in-flight transfer
                eng = nc.sync if gi % 2 == 0 else nc.gpsimd
                eng.dma_start(out=sim_pm[:, mo:mo + msz, :], in_=out_sb[:, :msz, :])

            groups = [1, 2, 3, 3] + [3] * 7 + [2]
            mo = 0
            for gi, msz in enumerate(groups):
                out_group(mo, msz, gi)
                mo += msz
            assert mo == NM

        if loop_iters == 1:
            body()
        else:
            with tc.For_i(0, loop_iters, 1):
                body()


_NC_CACHE = {}


def _np_bf16(x):
    return np.asarray(x).astype(mybir.dt.np(BF16))


def _timing_input_arrays(cur, ref):
    """Map device-input tensor names -> per-batch host arrays for test.py."""
    return {"cur": _np_bf16(cur), "ref": _np_bf16(ref)}


def _get_nc(loop_iters=1):
    key = ("nc", loop_iters)
    if key not in _NC_CACHE:
        nc = bacc.Bacc("TRN2", target_bir_lowering=False, debug=False)
        cur_d = nc.dram_tensor("cur", [C, HW], BF16, kind="ExternalInput")
        ref_d = nc.dram_tensor("ref", [C, HW], BF16, kind="ExternalInput")
        sim_d = nc.dram_tensor("sim", [HW, HW], BF16, kind="ExternalOutput")
        with tile.TileContext(nc) as tc:
            _kernel_body(tc, cur_d.ap(), ref_d.ap(), sim_d.ap(), loop_iters=loop_iters)
        nc.compile()
        _NC_CACHE[key] = nc
    return _NC_CACHE[key]


def kernel(ref_features, cur_features, _run_kwargs=None):
    ref_np = _np_bf16(
        np.ascontiguousarray(np.asarray(ref_features, dtype=np.float32)).reshape(
            B, C, HW
        )
    )
    cur_np = _np_bf16(
        np.ascontiguousarray(np.asarray(cur_features, dtype=np.float32)).reshape(
            B, C, HW
        )
    )
    nc = _get_nc()
    in_maps = [{"cur": cur_np[b], "ref": ref_np[b]} for b in range(B)]
    res = bass_utils.run_bass_kernel_spmd(
        nc, in_maps, core_ids=list(range(B)), **(_run_kwargs or {})
    )
    out = np.stack(
        [np.asarray(res.results[b]["sim"]).astype(np.float32) for b in range(B)],
        axis=0,
    )
    if _run_kwargs is not None:
        _NC_CACHE["last_results"] = res
    return out.reshape(B, H, W, H, W)
